# revision 29
# baseline (speedup 1.0000x reference)
"""Causal self-attention (B=2, T=4096, C=768, H=12) on 8 trn2 NeuronCores.

Sharding: core c handles batch b = c//4 and the 3 heads of head-group
hg = c%4 (tensor parallel over heads, data parallel over batch).  Each core
computes the qkv projection for its heads, causal attention, and a partial
output projection; the host sums the 4 per-head-group partials per batch.

Device notes:
  - Matmul inputs are bf16 (fp32 matmul runs LOW_HIGH = 2 PE passes);
    accumulation is fp32 in PSUM.  Host pre-transposes all operands so the
    contraction dim is on SBUF partitions.
  - Scores are computed transposed (S^T[tk, tq] = K Q^T) so P V needs no
    on-chip transposes.  The d=64 contraction uses 2x row tiling: two heads
    (partitions 0-63 / 64-127) run concurrently in the 64x128 PE mode.
  - The kernel is ACT-bound: softmax exp must run on the scalar engine at
    1 elem/cycle/lane, ~241us for the 28M scores this core owns.  The whole
    schedule is therefore a single fine-grained pipeline that keeps ACT fed:
    per q-chunk, attention runs as a stream of groups (4 S^T matmuls -> 2
    exps -> 4 PV matmuls, PV lagging one group), and the qkv projection
    chunks + output-projection blocks are pumped between groups as filler
    PE work.  PSUM: sp 2x[128,1024] (score staging), pso 2x[128,512]
    (PV accumulators: h0/h1 pair then h2), shr 2x[128,512] (qkv/proj).
  - Softmax denominator comes from an all-ones 65th column appended to V;
    normalization broadcasts the reciprocal row across partitions on GpSimd.
    Softmax skips the max subtraction: scores are ~N(0,1), exp is fp32-safe.
"""

from collections import deque

import ml_dtypes
import numpy as np

import concourse.bass as bass
import concourse.mybir as mybir
import concourse.tile as tile
from concourse import bacc

B, T, C, H, HD = 2, 4096, 768, 12, 64
F32 = mybir.dt.float32
BF16 = mybir.dt.bfloat16
N_CORES = 8
AF = mybir.ActivationFunctionType


def build_nc(seq_len: int = T) -> bass.Bass:
    assert seq_len % 512 == 0
    TCH = seq_len // 512   # 512-wide t-chunks
    TB = seq_len // 128    # 128-wide t-blocks

    nc = bacc.Bacc(num_devices=N_CORES)

    xT = nc.dram_tensor("xT", (C, seq_len), BF16, kind="ExternalInput").ap()
    wqkT = nc.dram_tensor("wqkT", (C, 384), BF16, kind="ExternalInput").ap()
    wvT = nc.dram_tensor("wvT", (C, 192), BF16, kind="ExternalInput").ap()
    wpT = nc.dram_tensor("wpT", (192, C), BF16, kind="ExternalInput").ap()
    out = nc.dram_tensor("out", (seq_len, C), F32, kind="ExternalOutput").ap()

    with tile.TileContext(nc) as tc:
        with (
            tc.tile_pool(name="const", bufs=1) as const,
            tc.tile_pool(name="persist", bufs=1) as persist,
            tc.tile_pool(name="xt", bufs=3) as xtpool,
            tc.tile_pool(name="p", bufs=8) as ppool,
            tc.tile_pool(name="small", bufs=4) as spool,
            tc.tile_pool(name="osb", bufs=3) as osbpool,
            tc.tile_pool(name="ps", bufs=2, space="PSUM") as psum,
        ):
            # ---- constants / weights ----
            # wqk first: the first qkv chain needs it + the first x chunk
            wqk_sb = const.tile([128, 6, 384], BF16, tag="wqk")
            nc.sync.dma_start(wqk_sb, wqkT.rearrange("(cc p) o -> p cc o", p=128))
            # warmup scratch: built on gpsimd (no DMA wait) so dummy matmuls
            # can warm the PE HAM clock gate while the input DMAs run
            wup = const.tile([128, 512], BF16, tag="wup")
            nc.gpsimd.memset(wup, 0.0)

            # emask[i, j] = 1.0 if j >= i + 384 else 0.0 (causal masks for the
            # 4 partially-masked k-blocks of each 512-wide q-chunk)
            emask = const.tile([128, 896], BF16, tag="emask")
            nc.gpsimd.memset(emask, 1.0)
            nc.gpsimd.affine_select(
                out=emask, in_=emask,
                compare_op=mybir.AluOpType.is_ge,
                fill=0.0, base=-384, pattern=[[1, 896]], channel_multiplier=-1,
            )

            # ---- persistent activations ----
            # qT/kT: slab h holds head h's 64 dims duplicated on both
            # partition halves, so every S^T half-group can run its two
            # matmuls on complementary row-halves (rows 0-63 for kb0, rows
            # 64-127 for kb0+1) — adjacent matmuls always pair in the
            # 64x128 row-tiled PE mode no matter what runs between groups.
            qT_sb = persist.tile([128, 3, seq_len], BF16, tag="qT")
            kT_sb = persist.tile([128, 3, seq_len], BF16, tag="kT")
            # v per head: [t-partition, kb, 64 dims + ones column]
            v_sb = [
                persist.tile([128, TB, 65], BF16, tag=f"v{h}", name=f"v{h}")
                for h in range(3)
            ]
            for h in range(3):
                nc.gpsimd.memset(v_sb[h][:, :, 64], 1.0)
            # attention output, transposed: chunk0 = [h0 | h1], chunk1 = [h2 | 0]
            outT_sb = persist.tile([128, 2, seq_len], BF16, tag="outT")
            nc.gpsimd.memset(outT_sb[64:128, 1, :], 0.0)

            # ---- background work generators (filler PE work) ----
            def qkv_gen(tci):
                tcs = slice(tci * 512, (tci + 1) * 512)
                xt = xtpool.tile([128, 6, 512], BF16, tag="xt", name=f"xt{tci}")
                for cc in range(6):
                    nc.sync.dma_start(
                        xt[:, cc, :], xT[cc * 128:(cc + 1) * 128, tcs]
                    )
                yield
                # q/k channels: m0=[q_h0|q_h1], m1=[k_h0|k_h1], m2=[q_h2|k_h2].
                # Land each head's rows on the lane-aligned half of its slab,
                # then duplicate to the other half via SBUF->SBUF DMA.
                for m in range(3):
                    ps = psum.tile([128, 512], F32, tag="shr", name=f"ps{tci}_{m}")
                    for cc in range(6):
                        nc.tensor.matmul(
                            ps,
                            lhsT=wqk_sb[:, cc, m * 128:(m + 1) * 128],
                            rhs=xt[:, cc, :],
                            start=(cc == 0), stop=(cc == 5),
                        )
                    dst_lo, dst_hi = (
                        (qT_sb[0:64, 0, tcs], qT_sb[64:128, 1, tcs]),
                        (kT_sb[0:64, 0, tcs], kT_sb[64:128, 1, tcs]),
                        (qT_sb[0:64, 2, tcs], kT_sb[64:128, 2, tcs]),
                    )[m]
                    nc.vector.tensor_copy(dst_lo, ps[0:64, :])
                    nc.vector.tensor_copy(dst_hi, ps[64:128, :])
                    if m == 0:
                        nc.sync.dma_start(qT_sb[64:128, 0, tcs], qT_sb[0:64, 0, tcs])
                        nc.sync.dma_start(qT_sb[0:64, 1, tcs], qT_sb[64:128, 1, tcs])
                    elif m == 1:
                        nc.sync.dma_start(kT_sb[64:128, 0, tcs], kT_sb[0:64, 0, tcs])
                        nc.sync.dma_start(kT_sb[0:64, 1, tcs], kT_sb[64:128, 1, tcs])
                    else:
                        nc.sync.dma_start(qT_sb[64:128, 2, tcs], qT_sb[0:64, 2, tcs])
                        nc.sync.dma_start(kT_sb[0:64, 2, tcs], kT_sb[64:128, 2, tcs])
                    yield
                # v channels
                for tb in range(4):
                    psv = psum.tile([128, 512], F32, tag="shr", name=f"psv{tci}_{tb}")
                    for cc in range(6):
                        nc.tensor.matmul(
                            psv[:, :192],
                            lhsT=xt[:, cc, tb * 128:(tb + 1) * 128],
                            rhs=wv_sb[:, cc, :],
                            start=(cc == 0), stop=(cc == 5),
                        )
                    for h in range(3):
                        nc.vector.tensor_copy(
                            v_sb[h][:, tci * 4 + tb, 0:64],
                            psv[:, 64 * h:64 * h + 64],
                        )
                    yield

            def proj_gen(qc):
                for tb in range(4 * qc, 4 * qc + 4):
                    tbs = slice(tb * 128, (tb + 1) * 128)
                    ob = osbpool.tile([128, 768], F32, tag="osb", name=f"ob{tb}")
                    for n0, nsz in ((0, 512), (512, 256)):
                        pp = psum.tile([128, 512], F32, tag="shr",
                                       name=f"pp{tb}_{n0}")
                        nc.tensor.matmul(
                            pp[:, :nsz],
                            lhsT=outT_sb[:, 0, tbs],
                            rhs=wp0_sb[:, n0:n0 + nsz],
                            start=True, stop=False,
                        )
                        nc.tensor.matmul(
                            pp[:, :nsz],
                            lhsT=outT_sb[:, 1, tbs],
                            rhs=wp1_sb[:, n0:n0 + nsz],
                            start=False, stop=True,
                        )
                        nc.vector.tensor_copy(ob[:, n0:n0 + nsz], pp[:, :nsz])
                        yield
                    nc.sync.dma_start(out[tbs, :], ob)
                    yield

            background = deque()  # entries: (kind, idx, generator)

            def pump(n=1):
                for _ in range(n):
                    while background:
                        try:
                            next(background[0][2])
                            break
                        except StopIteration:
                            background.popleft()

            def force_qkv(tci_needed):
                while any(k == 'qkv' and i <= tci_needed
                          for (k, i, _) in background):
                    pump(1)

            # warm up the PE while the initial DMAs land
            for i in range(8):
                wps = psum.tile([128, 512], F32, tag="shr", name=f"wps{i}")
                nc.tensor.matmul(wps, lhsT=wup[:, 0:128], rhs=wup,
                                 start=True, stop=True)

            # upfront: qkv(0) fully; qkv(1) backgrounded into attn(0).
            # Issue qkv(0)'s x DMAs right after wqk so the first projection
            # chain starts ASAP; the remaining consts load behind them.
            g0 = qkv_gen(0)
            next(g0)
            wv_sb = const.tile([128, 6, 192], BF16, tag="wv")
            nc.sync.dma_start(wv_sb, wvT.rearrange("(cc p) o -> p cc o", p=128))
            wp0_sb = const.tile([128, 768], BF16, tag="wp0")
            nc.sync.dma_start(wp0_sb, wpT[0:128, :])
            # zero-pad wp1 to 128 partitions so the proj matmul stays K=128
            wp1_sb = const.tile([128, 768], BF16, tag="wp1")
            nc.gpsimd.memset(wp1_sb[64:128, :], 0.0)
            nc.sync.dma_start(wp1_sb[0:64, :], wpT[128:192, :])
            for _ in range(3):  # q/k chains of chunk 0 (attn(0) needs these)
                next(g0)
            if TCH >= 2:
                # v-chains of chunk 0 + chunk 1 finish in the background
                # ('qkvv' so force_qkv does not drain them at attn(0) start)
                background.append(('qkvv', 0, g0))
                background.append(('qkv', 1, qkv_gen(1)))
            else:
                for _ in g0:
                    pass

            # ---- attention: one fine-grained pipeline per q-chunk ----
            # proj(j) is deferrable filler work: schedule it into the late,
            # ACT-slack-rich q-chunks (early chunks are PE-bound already)
            proj_sched = {TCH - 3: [0, 1], TCH - 2: [2, 3, 4], TCH - 1: [5, 6]}
            for qc in range(TCH):
                # just-in-time qkv: chunk qc+1 issues during attn(qc), so the
                # PE-deficit early windows are not overloaded with lookahead
                if qc + 1 < TCH and qc >= 1:
                    background.append(('qkv', qc + 1, qkv_gen(qc + 1)))
                for j in proj_sched.get(qc, []):
                    if 0 <= j < TCH - 1:
                        background.append(('proj', j, proj_gen(j)))
                force_qkv(qc)

                qcs = slice(qc * 512, (qc + 1) * 512)
                nkb = 4 * (qc + 1)
                half = nkb // 2
                ptmap = {}
                psos = {}

                # half-groups: (head, kb0) covering k-blocks kb0 (PE rows
                # 0-63) and kb0+1 (rows 64-127).  The two matmuls of a group
                # run on complementary row-halves -> they pair in the 64x128
                # row-tiled mode, write one sp tile, and one exp serves both
                # (so the 2-deep sp ring has 2 exps of slack).  Heads run
                # sequentially so only 2 PV accumulators are live at once.
                hgs = [(h, kb0) for h in range(3)
                       for kb0 in range(0, nkb, 2)]

                def issue_st(hg, qc=qc, qcs=qcs, ptmap=ptmap):
                    h, kb0 = hg
                    sp = psum.tile([128, 1024], F32, tag="sp", name="sp")
                    for t in (0, 1):
                        kb = kb0 + t
                        base = 64 * t
                        nc.tensor.matmul(
                            sp[:, t * 512:(t + 1) * 512],
                            lhsT=kT_sb[base:base + 64, h,
                                       kb * 128:(kb + 1) * 128],
                            rhs=qT_sb[base:base + 64, h, qcs],
                            start=True, stop=True,
                        )
                    pt = ppool.tile([128, 1024], BF16, tag="p", name="pt")
                    nc.scalar.activation(pt, sp, AF.Exp, scale=0.125)
                    for t in (0, 1):
                        kb = kb0 + t
                        hs = slice(t * 512, (t + 1) * 512)
                        poff = kb - 4 * qc
                        if poff >= 0:  # partially-causal diagonal block
                            nc.vector.tensor_mul(
                                pt[:, hs], pt[:, hs],
                                emask[:, 384 - 128 * poff: 896 - 128 * poff],
                            )
                        ptmap[(h, kb)] = (pt, hs)

                def evac(h, qc=qc, qcs=qcs, psos=psos):
                    # softmax denominator sits on psum partition 64; spread
                    # the reciprocal over 64 partitions via SBUF DMA so the
                    # iterative DVE reciprocal runs ~64x faster
                    ocp = spool.tile([128, 512], F32, tag="ocp",
                                     name=f"ocp{h}")
                    nc.vector.tensor_copy(ocp[0:65, :], psos[h][0:65, :])
                    slab = 1 if h == 2 else 0
                    lsplit = spool.tile([64, 8], F32, tag="lsplit")
                    nc.sync.dma_start(lsplit, ocp[64:65, :])
                    lrec = spool.tile([64, 8], F32, tag="lrec")
                    nc.vector.reciprocal(lrec, lsplit)
                    lrow = spool.tile([1, 512], F32, tag="lrow")
                    nc.sync.dma_start(lrow, lrec)
                    bc = spool.tile([64, 512], F32, tag="bc")
                    nc.gpsimd.partition_broadcast(bc, lrow)
                    if h == 1:
                        stg2 = spool.tile([64, 512], BF16, tag="stg2")
                        nc.vector.tensor_mul(stg2, ocp[0:64, :], bc)
                        nc.sync.dma_start(outT_sb[64:128, 0, qcs], stg2)
                    else:
                        nc.vector.tensor_mul(
                            outT_sb[0:64, slab, qcs], ocp[0:64, :], bc
                        )

                def issue_pv(hg, qc=qc, nkb=nkb, ptmap=ptmap, psos=psos):
                    h, kb0 = hg
                    if h not in psos:
                        psos[h] = psum.tile([128, 512], F32, tag="pso",
                                            name=f"pso{h}")
                    for t in (0, 1):
                        kb = kb0 + t
                        pt, hs = ptmap.pop((h, kb))
                        nc.tensor.matmul(
                            psos[h][0:65, :],
                            lhsT=v_sb[h][:, kb, :],
                            rhs=pt[:, hs],
                            start=(kb == 0), stop=(kb == nkb - 1),
                        )
                        if kb == nkb - 1:
                            evac(h)

                for i, hg in enumerate(hgs):
                    if qc == 0 and i < 2:
                        issue_st(hg)  # feed ACT before any filler work
                        pump(2)
                    else:
                        pump(2)
                        issue_st(hg)
                    if i >= 2:
                        issue_pv(hgs[i - 2])
                issue_pv(hgs[-2])
                issue_pv(hgs[-1])
                psos.clear()

            # tail: keep the PE clock gate warm through the final
            # normalization latency, then the last output projection
            for i in range(24):
                wps = psum.tile([128, 512], F32, tag="shr", name=f"wt{i}")
                nc.tensor.matmul(wps, lhsT=wup[:, 0:128], rhs=wup,
                                 start=True, stop=True)
            background.append(('proj', TCH - 1, proj_gen(TCH - 1)))
            while background:
                pump(1)

    nc.compile()
    return nc


_NC_CACHE: dict[int, bass.Bass] = {}


def get_nc(seq_len: int) -> bass.Bass:
    if seq_len not in _NC_CACHE:
        _NC_CACHE[seq_len] = build_nc(seq_len)
    return _NC_CACHE[seq_len]


def make_in_maps(x: np.ndarray, w_attn: np.ndarray, w_proj: np.ndarray):
    """Per-core input dicts. Core c: batch c//4, head group c%4 (3 heads)."""
    bf16 = ml_dtypes.bfloat16
    in_maps = []
    for c in range(N_CORES):
        b, hg = divmod(c, 4)
        q = w_attn[192 * hg: 192 * hg + 192]
        k = w_attn[768 + 192 * hg: 768 + 192 * hg + 192]
        v = w_attn[1536 + 192 * hg: 1536 + 192 * hg + 192]
        wqk = np.concatenate([q[0:128], k[0:128], q[128:192], k[128:192]], axis=0)
        in_maps.append({
            "xT": np.ascontiguousarray(x[b].T).astype(bf16),
            "wqkT": np.ascontiguousarray(wqk.T).astype(bf16),
            "wvT": np.ascontiguousarray(v.T).astype(bf16),
            "wpT": np.ascontiguousarray(
                w_proj[:, 192 * hg: 192 * hg + 192].T
            ).astype(bf16),
        })
    return in_maps


def run_on_cores(x, w_attn, w_proj, trace: bool = False):
    from concourse.bass_utils import run_bass_kernel_spmd

    x = np.asarray(x, dtype=np.float32)
    w_attn = np.asarray(w_attn, dtype=np.float32)
    w_proj = np.asarray(w_proj, dtype=np.float32)
    nc = get_nc(x.shape[1])
    in_maps = make_in_maps(x, w_attn, w_proj)
    res = run_bass_kernel_spmd(
        nc, in_maps, core_ids=list(range(N_CORES)), trace=trace
    )
    outs = [r["out"] for r in res.results]
    full = np.stack(
        [sum(outs[4 * b + hg] for hg in range(4)) for b in range(B)], axis=0
    )
    return full, res


def kernel(x, w_attn, w_proj):
    full, _ = run_on_cores(x, w_attn, w_proj, trace=False)
    return full


# revision 32
# speedup vs baseline: 1.0011x; 1.0011x over previous
"""Causal self-attention (B=2, T=4096, C=768, H=12) on 8 trn2 NeuronCores.

Sharding: core c handles batch b = c//4 and the 3 heads of head-group
hg = c%4 (tensor parallel over heads, data parallel over batch).  Each core
computes the qkv projection for its heads, causal attention, and a partial
output projection; the host sums the 4 per-head-group partials per batch.

Device notes:
  - Matmul inputs are bf16 (fp32 matmul runs LOW_HIGH = 2 PE passes);
    accumulation is fp32 in PSUM.  Host pre-transposes all operands so the
    contraction dim is on SBUF partitions.
  - Scores are computed transposed (S^T[tk, tq] = K Q^T) so P V needs no
    on-chip transposes.  The d=64 contraction uses 2x row tiling: two heads
    (partitions 0-63 / 64-127) run concurrently in the 64x128 PE mode.
  - The kernel is ACT-bound: softmax exp must run on the scalar engine at
    1 elem/cycle/lane, ~241us for the 28M scores this core owns.  The whole
    schedule is therefore a single fine-grained pipeline that keeps ACT fed:
    per q-chunk, attention runs as a stream of groups (4 S^T matmuls -> 2
    exps -> 4 PV matmuls, PV lagging one group), and the qkv projection
    chunks + output-projection blocks are pumped between groups as filler
    PE work.  PSUM: sp 2x[128,1024] (score staging), pso 2x[128,512]
    (PV accumulators: h0/h1 pair then h2), shr 2x[128,512] (qkv/proj).
  - Softmax denominator comes from an all-ones 65th column appended to V;
    normalization broadcasts the reciprocal row across partitions on GpSimd.
    Softmax skips the max subtraction: scores are ~N(0,1), exp is fp32-safe.
"""

from collections import deque

import ml_dtypes
import numpy as np

import concourse.bass as bass
import concourse.mybir as mybir
import concourse.tile as tile
from concourse import bacc

B, T, C, H, HD = 2, 4096, 768, 12, 64
F32 = mybir.dt.float32
BF16 = mybir.dt.bfloat16
N_CORES = 8
AF = mybir.ActivationFunctionType


def build_nc(seq_len: int = T) -> bass.Bass:
    assert seq_len % 512 == 0
    TCH = seq_len // 512   # 512-wide t-chunks
    TB = seq_len // 128    # 128-wide t-blocks

    nc = bacc.Bacc(num_devices=N_CORES)

    xT = nc.dram_tensor("xT", (C, seq_len), BF16, kind="ExternalInput").ap()
    wqkT = nc.dram_tensor("wqkT", (C, 384), BF16, kind="ExternalInput").ap()
    wvT = nc.dram_tensor("wvT", (C, 192), BF16, kind="ExternalInput").ap()
    wpT = nc.dram_tensor("wpT", (192, C), BF16, kind="ExternalInput").ap()
    out = nc.dram_tensor("out", (seq_len, C), F32, kind="ExternalOutput").ap()

    with tile.TileContext(nc) as tc:
        with (
            tc.tile_pool(name="const", bufs=1) as const,
            tc.tile_pool(name="persist", bufs=1) as persist,
            tc.tile_pool(name="xt", bufs=3) as xtpool,
            tc.tile_pool(name="p", bufs=8) as ppool,
            tc.tile_pool(name="small", bufs=4) as spool,
            tc.tile_pool(name="osb", bufs=3) as osbpool,
            tc.tile_pool(name="ps", bufs=2, space="PSUM") as psum,
        ):
            # ---- constants / weights ----
            # wqk first: the first qkv chain needs it + the first x chunk
            wqk_sb = const.tile([128, 6, 384], BF16, tag="wqk")
            nc.sync.dma_start(wqk_sb, wqkT.rearrange("(cc p) o -> p cc o", p=128))
            # warmup scratch: built on gpsimd (no DMA wait) so dummy matmuls
            # can warm the PE HAM clock gate while the input DMAs run
            wup = const.tile([128, 512], BF16, tag="wup")
            nc.gpsimd.memset(wup, 0.0)

            # emask[i, j] = 1.0 if j >= i + 384 else 0.0 (causal masks for the
            # 4 partially-masked k-blocks of each 512-wide q-chunk)
            emask = const.tile([128, 896], BF16, tag="emask")
            nc.gpsimd.memset(emask, 1.0)
            nc.gpsimd.affine_select(
                out=emask, in_=emask,
                compare_op=mybir.AluOpType.is_ge,
                fill=0.0, base=-384, pattern=[[1, 896]], channel_multiplier=-1,
            )

            # ---- persistent activations ----
            # qT/kT: slab h holds head h's 64 dims duplicated on both
            # partition halves, so every S^T half-group can run its two
            # matmuls on complementary row-halves (rows 0-63 for kb0, rows
            # 64-127 for kb0+1) — adjacent matmuls always pair in the
            # 64x128 row-tiled PE mode no matter what runs between groups.
            qT_sb = persist.tile([128, 3, seq_len], BF16, tag="qT")
            kT_sb = persist.tile([128, 3, seq_len], BF16, tag="kT")
            # v per head: [t-partition, kb, 64 dims + ones column]
            v_sb = [
                persist.tile([128, TB, 65], BF16, tag=f"v{h}", name=f"v{h}")
                for h in range(3)
            ]
            for h in range(3):
                nc.gpsimd.memset(v_sb[h][:, :, 64], 1.0)
            # attention output, transposed: chunk0 = [h0 | h1], chunk1 = [h2 | 0]
            outT_sb = persist.tile([128, 2, seq_len], BF16, tag="outT")
            nc.gpsimd.memset(outT_sb[64:128, 1, :], 0.0)

            # ---- background work generators (filler PE work) ----
            def qkv_gen(tci):
                tcs = slice(tci * 512, (tci + 1) * 512)
                xt = xtpool.tile([128, 6, 512], BF16, tag="xt", name=f"xt{tci}")
                for cc in range(6):
                    nc.sync.dma_start(
                        xt[:, cc, :], xT[cc * 128:(cc + 1) * 128, tcs]
                    )
                yield
                # q/k channels: m0=[q_h0|q_h1], m1=[k_h0|k_h1], m2=[q_h2|k_h2].
                # Land each head's rows on the lane-aligned half of its slab,
                # then duplicate to the other half via SBUF->SBUF DMA.
                for m in range(3):
                    ps = psum.tile([128, 512], F32, tag="shr", name=f"ps{tci}_{m}")
                    for cc in range(6):
                        nc.tensor.matmul(
                            ps,
                            lhsT=wqk_sb[:, cc, m * 128:(m + 1) * 128],
                            rhs=xt[:, cc, :],
                            start=(cc == 0), stop=(cc == 5),
                        )
                    dst_lo, dst_hi = (
                        (qT_sb[0:64, 0, tcs], qT_sb[64:128, 1, tcs]),
                        (kT_sb[0:64, 0, tcs], kT_sb[64:128, 1, tcs]),
                        (qT_sb[0:64, 2, tcs], kT_sb[64:128, 2, tcs]),
                    )[m]
                    nc.vector.tensor_copy(dst_lo, ps[0:64, :])
                    nc.vector.tensor_copy(dst_hi, ps[64:128, :])
                    if m == 0:
                        nc.sync.dma_start(qT_sb[64:128, 0, tcs], qT_sb[0:64, 0, tcs])
                        nc.sync.dma_start(qT_sb[0:64, 1, tcs], qT_sb[64:128, 1, tcs])
                    elif m == 1:
                        nc.sync.dma_start(kT_sb[64:128, 0, tcs], kT_sb[0:64, 0, tcs])
                        nc.sync.dma_start(kT_sb[0:64, 1, tcs], kT_sb[64:128, 1, tcs])
                    else:
                        nc.sync.dma_start(qT_sb[64:128, 2, tcs], qT_sb[0:64, 2, tcs])
                        nc.sync.dma_start(kT_sb[0:64, 2, tcs], kT_sb[64:128, 2, tcs])
                    yield
                # v channels
                for tb in range(4):
                    psv = psum.tile([128, 512], F32, tag="shr", name=f"psv{tci}_{tb}")
                    for cc in range(6):
                        nc.tensor.matmul(
                            psv[:, :192],
                            lhsT=xt[:, cc, tb * 128:(tb + 1) * 128],
                            rhs=wv_sb[:, cc, :],
                            start=(cc == 0), stop=(cc == 5),
                        )
                    for h in range(3):
                        nc.vector.tensor_copy(
                            v_sb[h][:, tci * 4 + tb, 0:64],
                            psv[:, 64 * h:64 * h + 64],
                        )
                    yield

            def proj_gen(qc):
                for tb in range(4 * qc, 4 * qc + 4):
                    tbs = slice(tb * 128, (tb + 1) * 128)
                    ob = osbpool.tile([128, 768], F32, tag="osb", name=f"ob{tb}")
                    for n0, nsz in ((0, 512), (512, 256)):
                        pp = psum.tile([128, 512], F32, tag="shr",
                                       name=f"pp{tb}_{n0}")
                        nc.tensor.matmul(
                            pp[:, :nsz],
                            lhsT=outT_sb[:, 0, tbs],
                            rhs=wp0_sb[:, n0:n0 + nsz],
                            start=True, stop=False,
                        )
                        nc.tensor.matmul(
                            pp[:, :nsz],
                            lhsT=outT_sb[:, 1, tbs],
                            rhs=wp1_sb[:, n0:n0 + nsz],
                            start=False, stop=True,
                        )
                        nc.vector.tensor_copy(ob[:, n0:n0 + nsz], pp[:, :nsz])
                        yield
                    nc.sync.dma_start(out[tbs, :], ob)
                    yield

            background = deque()  # entries: (kind, idx, generator)

            def pump(n=1):
                for _ in range(n):
                    while background:
                        try:
                            next(background[0][2])
                            break
                        except StopIteration:
                            background.popleft()

            def force_qkv(tci_needed):
                while any(k == 'qkv' and i <= tci_needed
                          for (k, i, _) in background):
                    pump(1)

            # warm up the PE while the initial DMAs land
            for i in range(8):
                wps = psum.tile([128, 512], F32, tag="shr", name=f"wps{i}")
                nc.tensor.matmul(wps, lhsT=wup[:, 0:128], rhs=wup,
                                 start=True, stop=True)
            # pre-zero the pt ring: split diagonal exps leave part of each
            # tile untouched, and the emask multiply must see finite values
            for i in range(8):
                zt = ppool.tile([128, 1024], BF16, tag="p", name=f"zt{i}")
                nc.gpsimd.memset(zt, 0.0)

            # upfront: qkv(0) fully; qkv(1) backgrounded into attn(0).
            # Issue qkv(0)'s x DMAs right after wqk so the first projection
            # chain starts ASAP; the remaining consts load behind them.
            g0 = qkv_gen(0)
            next(g0)
            wv_sb = const.tile([128, 6, 192], BF16, tag="wv")
            nc.sync.dma_start(wv_sb, wvT.rearrange("(cc p) o -> p cc o", p=128))
            wp0_sb = const.tile([128, 768], BF16, tag="wp0")
            nc.sync.dma_start(wp0_sb, wpT[0:128, :])
            # zero-pad wp1 to 128 partitions so the proj matmul stays K=128
            wp1_sb = const.tile([128, 768], BF16, tag="wp1")
            nc.gpsimd.memset(wp1_sb[64:128, :], 0.0)
            nc.sync.dma_start(wp1_sb[0:64, :], wpT[128:192, :])
            for _ in range(3):  # q/k chains of chunk 0 (attn(0) needs these)
                next(g0)
            if TCH >= 2:
                # v-chains of chunk 0 + chunk 1 finish in the background
                # ('qkvv' so force_qkv does not drain them at attn(0) start)
                background.append(('qkvv', 0, g0))
                background.append(('qkv', 1, qkv_gen(1)))
            else:
                for _ in g0:
                    pass

            # ---- attention: one fine-grained pipeline per q-chunk ----
            # proj(j) is deferrable filler work: schedule it into the late,
            # ACT-slack-rich q-chunks (early chunks are PE-bound already)
            proj_sched = {TCH - 3: [0, 1], TCH - 2: [2, 3, 4], TCH - 1: [5, 6]}
            for qc in range(TCH):
                if qc + 2 < TCH:
                    background.append(('qkv', qc + 2, qkv_gen(qc + 2)))
                for j in proj_sched.get(qc, []):
                    if 0 <= j < TCH - 1:
                        background.append(('proj', j, proj_gen(j)))
                force_qkv(qc)

                qcs = slice(qc * 512, (qc + 1) * 512)
                nkb = 4 * (qc + 1)
                half = nkb // 2
                ptmap = {}
                psos = {}

                # half-groups: (head, kb0) covering k-blocks kb0 (PE rows
                # 0-63) and kb0+1 (rows 64-127).  The two matmuls of a group
                # run on complementary row-halves -> they pair in the 64x128
                # row-tiled mode, write one sp tile, and one exp serves both
                # (so the 2-deep sp ring has 2 exps of slack).  Heads run
                # sequentially so only 2 PV accumulators are live at once.
                hgs = [(h, kb0) for h in range(3)
                       for kb0 in range(0, nkb, 2)]

                def issue_st(hg, qc=qc, qcs=qcs, ptmap=ptmap):
                    h, kb0 = hg
                    p0 = kb0 - 4 * qc
                    sp = psum.tile([128, 1024], F32, tag="sp", name="sp")
                    if p0 >= 2:
                        # deep diagonal half-group: columns tq < 128*poff are
                        # entirely above the causal boundary -- skip them in
                        # both the S^T matmuls and the exp (the emask multiply
                        # still zeroes the stale pt region afterwards)
                        for t in (0, 1):
                            kb = kb0 + t
                            lo = 128 * (p0 + t)
                            nc.tensor.matmul(
                                sp[:, t * 512 + lo:(t + 1) * 512],
                                lhsT=kT_sb[64 * t:64 * t + 64, h,
                                           kb * 128:(kb + 1) * 128],
                                rhs=qT_sb[64 * t:64 * t + 64, h,
                                          qc * 512 + lo:(qc + 1) * 512],
                                start=True, stop=True,
                            )
                        pt = ppool.tile([128, 1024], BF16, tag="p", name="pt")
                        for t in (0, 1):
                            lo = 128 * (p0 + t)
                            nc.scalar.activation(
                                pt[:, t * 512 + lo:(t + 1) * 512],
                                sp[:, t * 512 + lo:(t + 1) * 512],
                                AF.Exp, scale=0.125,
                            )
                    else:
                        for t in (0, 1):
                            kb = kb0 + t
                            nc.tensor.matmul(
                                sp[:, t * 512:(t + 1) * 512],
                                lhsT=kT_sb[64 * t:64 * t + 64, h,
                                           kb * 128:(kb + 1) * 128],
                                rhs=qT_sb[64 * t:64 * t + 64, h, qcs],
                                start=True, stop=True,
                            )
                        pt = ppool.tile([128, 1024], BF16, tag="p", name="pt")
                        nc.scalar.activation(pt, sp, AF.Exp, scale=0.125)
                    for t in (0, 1):
                        kb = kb0 + t
                        hs = slice(t * 512, (t + 1) * 512)
                        poff = kb - 4 * qc
                        if poff >= 0:  # partially-causal diagonal block
                            nc.vector.tensor_mul(
                                pt[:, hs], pt[:, hs],
                                emask[:, 384 - 128 * poff: 896 - 128 * poff],
                            )
                        ptmap[(h, kb)] = (pt, hs)

                def evac(h, qc=qc, qcs=qcs, psos=psos):
                    # softmax denominator sits on psum partition 64; spread
                    # the reciprocal over 64 partitions via SBUF DMA so the
                    # iterative DVE reciprocal runs ~64x faster
                    ocp = spool.tile([128, 512], F32, tag="ocp",
                                     name=f"ocp{h}")
                    nc.vector.tensor_copy(ocp[0:65, :], psos[h][0:65, :])
                    slab = 1 if h == 2 else 0
                    lsplit = spool.tile([64, 8], F32, tag="lsplit")
                    nc.sync.dma_start(lsplit, ocp[64:65, :])
                    lrec = spool.tile([64, 8], F32, tag="lrec")
                    nc.vector.reciprocal(lrec, lsplit)
                    lrow = spool.tile([1, 512], F32, tag="lrow")
                    nc.sync.dma_start(lrow, lrec)
                    bc = spool.tile([64, 512], F32, tag="bc")
                    nc.gpsimd.partition_broadcast(bc, lrow)
                    if h == 1:
                        stg2 = spool.tile([64, 512], BF16, tag="stg2")
                        nc.vector.tensor_mul(stg2, ocp[0:64, :], bc)
                        nc.sync.dma_start(outT_sb[64:128, 0, qcs], stg2)
                    else:
                        nc.vector.tensor_mul(
                            outT_sb[0:64, slab, qcs], ocp[0:64, :], bc
                        )

                def issue_pv(hg, qc=qc, nkb=nkb, ptmap=ptmap, psos=psos):
                    h, kb0 = hg
                    if h not in psos:
                        psos[h] = psum.tile([128, 512], F32, tag="pso",
                                            name=f"pso{h}")
                    for t in (0, 1):
                        kb = kb0 + t
                        pt, hs = ptmap.pop((h, kb))
                        nc.tensor.matmul(
                            psos[h][0:65, :],
                            lhsT=v_sb[h][:, kb, :],
                            rhs=pt[:, hs],
                            start=(kb == 0), stop=(kb == nkb - 1),
                        )
                        if kb == nkb - 1:
                            evac(h)

                for i, hg in enumerate(hgs):
                    if qc == 0 and i < 2:
                        issue_st(hg)  # feed ACT before any filler work
                        pump(2)
                    else:
                        pump(2)
                        issue_st(hg)
                    if i >= 2:
                        issue_pv(hgs[i - 2])
                issue_pv(hgs[-2])
                issue_pv(hgs[-1])
                psos.clear()

            # tail: keep the PE clock gate warm through the final
            # normalization latency, then the last output projection
            for i in range(24):
                wps = psum.tile([128, 512], F32, tag="shr", name=f"wt{i}")
                nc.tensor.matmul(wps, lhsT=wup[:, 0:128], rhs=wup,
                                 start=True, stop=True)
            background.append(('proj', TCH - 1, proj_gen(TCH - 1)))
            while background:
                pump(1)

    nc.compile()
    return nc


_NC_CACHE: dict[int, bass.Bass] = {}


def get_nc(seq_len: int) -> bass.Bass:
    if seq_len not in _NC_CACHE:
        _NC_CACHE[seq_len] = build_nc(seq_len)
    return _NC_CACHE[seq_len]


def make_in_maps(x: np.ndarray, w_attn: np.ndarray, w_proj: np.ndarray):
    """Per-core input dicts. Core c: batch c//4, head group c%4 (3 heads)."""
    bf16 = ml_dtypes.bfloat16
    in_maps = []
    for c in range(N_CORES):
        b, hg = divmod(c, 4)
        q = w_attn[192 * hg: 192 * hg + 192]
        k = w_attn[768 + 192 * hg: 768 + 192 * hg + 192]
        v = w_attn[1536 + 192 * hg: 1536 + 192 * hg + 192]
        wqk = np.concatenate([q[0:128], k[0:128], q[128:192], k[128:192]], axis=0)
        in_maps.append({
            "xT": np.ascontiguousarray(x[b].T).astype(bf16),
            "wqkT": np.ascontiguousarray(wqk.T).astype(bf16),
            "wvT": np.ascontiguousarray(v.T).astype(bf16),
            "wpT": np.ascontiguousarray(
                w_proj[:, 192 * hg: 192 * hg + 192].T
            ).astype(bf16),
        })
    return in_maps


def run_on_cores(x, w_attn, w_proj, trace: bool = False):
    from concourse.bass_utils import run_bass_kernel_spmd

    x = np.asarray(x, dtype=np.float32)
    w_attn = np.asarray(w_attn, dtype=np.float32)
    w_proj = np.asarray(w_proj, dtype=np.float32)
    nc = get_nc(x.shape[1])
    in_maps = make_in_maps(x, w_attn, w_proj)
    res = run_bass_kernel_spmd(
        nc, in_maps, core_ids=list(range(N_CORES)), trace=trace
    )
    outs = [r["out"] for r in res.results]
    full = np.stack(
        [sum(outs[4 * b + hg] for hg in range(4)) for b in range(B)], axis=0
    )
    return full, res


def kernel(x, w_attn, w_proj):
    full, _ = run_on_cores(x, w_attn, w_proj, trace=False)
    return full
